# revision 18
# baseline (speedup 1.0000x reference)
"""Distributed Trainium2 kernel for nn_Attention_31104153157828.

Computation (B=16, S=2048, D=1024):
    fac1 = k @ W                     [B,S,D]
    fac2 = (q @ U)[:, None, :]       [B,1,D]
    t    = tanh(fac1 + fac2)
    s    = einsum('bsd,bse->bde', v, t)      [B,D,D]
    attn = softmax(s, axis=0)                 (softmax over BATCH)
    out  = einsum('bsd,bde->bse', v, attn)   [B,S,D]

Sharding: data-parallel over batch, 2 batches per core on 8 cores. The
batch-axis softmax needs a cross-core AllReduce of max and sum(exp) over
the [D,D] logits.  All matmul operands are bf16 (measured end-to-end
rel-err ~1.2e-2 vs the 2e-2 gate); accumulation is fp32 in PSUM; the
softmax itself runs in fp32.

The kernel is pipelined by e-half so the 4 AllReduces hide under PE work:
    A(h0) -> B(h0) -> AR-max(h0)
    A(h1) || (exp(h0) + AR-sum(h0))
    B(h1) || (AR-max(h1))
    C(h0) || (exp(h1) + AR-sum(h1))
    C(h1)
k^T and v^T tiles are produced by xbar DMA transpose from bf16 DRAM
scratch copies (no PE transposes, no DVE copies).  A/B/C all accumulate
kc-outer/m-inner across 8 persistent PSUM banks (long accumulation runs
keep the PE HAM clock warm); a tunable block of throwaway matmuls
bridges the remaining AR-bound bubble before C(h1).
"""
import numpy as np
import concourse.bass as bass
import concourse.bacc as bacc
import concourse.tile as tile
import concourse.mybir as mybir
from concourse.bass_utils import run_bass_kernel_spmd

F32 = mybir.dt.float32
F32R = mybir.dt.float32r
BF16 = mybir.dt.bfloat16
AF = mybir.ActivationFunctionType

B, S, D = 16, 2048, 1024
N_CORES = 8
BL = B // N_CORES          # local batches per core = 2
M_T = S // 128             # 16 s-tiles
KC = D // 128              # 8 contraction chunks (d)
EH = 2                     # e halves of 512
ARC = 4                    # softmax chunks (pairs of d-tiles)
N_DUMMY = 96               # HAM-warming filler matmuls before stage C(h1)
RG = [list(range(N_CORES))]


def build():
    nc = bacc.Bacc("TRN2", target_bir_lowering=False, debug=False,
                   num_devices=N_CORES)

    q2 = nc.dram_tensor("q2", [BL, D], F32, kind="ExternalInput")
    k2 = nc.dram_tensor("k2", [BL, S, D], F32, kind="ExternalInput")
    v2 = nc.dram_tensor("v2", [BL, S, D], F32, kind="ExternalInput")
    Wd = nc.dram_tensor("W", [D, D], F32, kind="ExternalInput")
    Ud = nc.dram_tensor("U", [D, D], F32, kind="ExternalInput")
    out2 = nc.dram_tensor("out", [BL, S, D], F32, kind="ExternalOutput")

    k_bf = nc.dram_tensor("k_bf", [BL, S, D], BF16)
    v_bf = nc.dram_tensor("v_bf", [BL, S, D], BF16)

    mx_in = [nc.dram_tensor(f"mx_in{h}", [128, KC, 512], BF16) for h in range(EH)]
    mx_out = [nc.dram_tensor(f"mx_out{h}", [128, KC, 512], BF16) for h in range(EH)]
    sm_in = [nc.dram_tensor(f"sm_in{h}", [128, KC, 512], BF16) for h in range(EH)]
    sm_out = [nc.dram_tensor(f"sm_out{h}", [128, KC, 512], BF16) for h in range(EH)]

    warm_in = nc.dram_tensor("warm_in", [128, 16], F32)
    warm_out = nc.dram_tensor("warm_out", [128, 16], F32)
    warm_out2 = nc.dram_tensor("warm_out2", [128, 16], F32)

    ones_d = nc.inline_tensor(np.ones((1, 128), np.float32), name="ones1")

    with tile.TileContext(nc) as tc:
        with tc.tile_pool(name="rp", bufs=1) as rp:
            # ---- residents: W (bf16), fac2, ones ----
            W_b = rp.tile([128, KC, D], BF16, name="W_b")
            ones_b = rp.tile([1, 128], BF16, name="ones_b")
            fac2 = rp.tile([1, BL, D], BF16, name="fac2")
            wtile = rp.tile([128, 16], F32, name="wtile")

            # gpsimd queue, in priority order: q/U (gate fac2), W, k_bf
            # chunks (gate stage A), warm-AR plumbing, v_bf
            with (
                tc.tile_pool(name="f2", bufs=2) as f2p,
                tc.tile_pool(name="f2u", bufs=1) as f2u,
                tc.tile_pool(name="f2ps", bufs=2, space="PSUM") as f2ps,
            ):
                U_r = f2u.tile([128, KC, D], F32R, name="U_r")
                nc.gpsimd.dma_start(
                    U_r[:], Ud.ap().rearrange("(kc p) e -> p kc e", p=128))
                qcols = []
                for b in range(BL):
                    qcol = f2p.tile([128, KC], F32R, tag="qcol", name=f"qcol{b}")
                    nc.gpsimd.dma_start(
                        qcol[:], q2.ap()[b].rearrange("(kc p) -> p kc", p=128))
                    qcols.append(qcol)
                nc.gpsimd.dma_start(
                    W_b[:], Wd.ap().rearrange("(kc p) e -> p kc e", p=128))
                nc.gpsimd.dma_start(ones_b[:], ones_d.ap())
                # k_bf pre-cast in chunks so A(b0,m0) starts early
                for b in range(BL):
                    for c4 in range(4):
                        nc.gpsimd.dma_start(
                            k_bf.ap()[b, c4 * 512:(c4 + 1) * 512],
                            k2.ap()[b, c4 * 512:(c4 + 1) * 512])
                # warm up the collective machinery (first AR pays ~70us)
                nc.gpsimd.dma_start(wtile[:], Wd.ap()[0:128, 0:16])
                nc.gpsimd.dma_start(warm_in.ap(), wtile[:])
                ar_w1 = nc.gpsimd.collective_compute(
                    "AllReduce", mybir.AluOpType.max, replica_groups=RG,
                    ins=[warm_in.ap().opt()], outs=[warm_out.ap().opt()])
                ar_w2 = nc.gpsimd.collective_compute(
                    "AllReduce", mybir.AluOpType.add, replica_groups=RG,
                    ins=[warm_out.ap().opt()], outs=[warm_out2.ap().opt()])
                nc.gpsimd.dma_start(v_bf.ap()[0], v2.ap()[0])
                nc.gpsimd.dma_start(v_bf.ap()[1], v2.ap()[1])

                # fac2 = q @ U  (f32r inputs, bf16 out)
                for b in range(BL):
                    for h in range(EH):
                        ps = f2ps.tile([1, 512], F32, tag="f2ps",
                                       name=f"f2ps{b}_{h}")
                        for kc in range(KC):
                            nc.tensor.matmul(ps[:], qcols[b][:, kc:kc + 1],
                                             U_r[:, kc, h * 512:(h + 1) * 512],
                                             start=(kc == 0), stop=(kc == KC - 1))
                        nc.scalar.copy(fac2[0:1, b, h * 512:(h + 1) * 512], ps[:])

            # ---- per-half t tiles (bf16), slots reused across halves ----
            tp_cm = tc.tile_pool(name="tp", bufs=1)
            tp = tp_cm.__enter__()

            # shared 8-bank PSUM pool used by A groups, B, C and dummies
            ps8_cm = tc.tile_pool(name="ps8", bufs=1, space="PSUM")
            ps8 = ps8_cm.__enter__()

            ktp_cm = tc.tile_pool(name="ktp", bufs=12)
            ktp = ktp_cm.__enter__()

            sm_cm = tc.tile_pool(name="smx", bufs=2)
            smx = sm_cm.__enter__()
            sfp_cm = tc.tile_pool(name="sfp", bufs=2)
            sfp = sfp_cm.__enter__()

            bp_cm = tc.tile_pool(name="bp", bufs=3)
            bp = bp_cm.__enter__()

            def stage_a(h):
                # t[:, :, h] = tanh(k @ W[:, h] + fac2[h]) for both batches;
                # kc-outer / m-inner over 8 PSUM banks
                t_h = []
                for b in range(BL):
                    t_b = tp.tile([128, M_T, 512], BF16, tag=f"t{b}",
                                  name=f"t{h}_{b}")
                    for mg in (0, 8):
                        kts = []
                        for j in range(8):
                            m = mg + j
                            kt = ktp.tile([128, KC, 128], BF16, tag="kt",
                                          name=f"kt{h}_{b}_{m}")
                            nc.sync.dma_start(
                                kt[:], k_bf.ap()[b, m * 128:(m + 1) * 128, :],
                                transpose=True)
                            kts.append(kt)
                        pss = [ps8.tile([128, 512], F32, tag=f"ps{j}",
                                        name=f"aps{h}_{b}_{mg}_{j}")
                               for j in range(8)]
                        for kc in range(KC):
                            for j in range(8):
                                nc.tensor.matmul(
                                    pss[j][:], kts[j][:, kc, :],
                                    W_b[:, kc, h * 512:(h + 1) * 512],
                                    start=(kc == 0), stop=False)
                        for j in range(8):
                            nc.tensor.matmul(
                                pss[j][:], ones_b[:],
                                fac2[0:1, b, h * 512:(h + 1) * 512],
                                start=False, stop=True)
                            nc.scalar.activation(
                                t_b[:, mg + j, :], pss[j][:], AF.Tanh)
                    t_h.append(t_b)
                return t_h

            def stage_b(h, t_h):
                # s[d,e] = sum_s v[s,d] t[s,e]: v stationary, t moving;
                # 8 d-banks accumulate across the full m loop per batch
                s_h = []
                for b in range(BL):
                    psb = [ps8.tile([128, 512], F32, tag=f"ps{dt}",
                                    name=f"bps{h}_{b}_{dt}") for dt in range(KC)]
                    for m in range(M_T):
                        vslab = bp.tile([128, D], BF16, tag="vslab",
                                        name=f"vslab{h}_{b}_{m}")
                        nc.sync.dma_start(
                            vslab[:], v_bf.ap()[b, m * 128:(m + 1) * 128, :])
                        for dt in range(KC):
                            nc.tensor.matmul(
                                psb[dt][:],
                                vslab[:, dt * 128:(dt + 1) * 128],
                                t_h[b][:, m, :],
                                start=(m == 0), stop=(m == M_T - 1))
                    s_b = sfp.tile([128, KC, 512], F32, tag=f"s{b}",
                                   name=f"s{h}_{b}")
                    for dt in range(KC):
                        nc.vector.tensor_copy(s_b[:, dt, :], psb[dt][:])
                    s_h.append(s_b)
                return s_h

            def local_max(h, s_h):
                for c in range(ARC):
                    dsl = slice(2 * c, 2 * c + 2)
                    mx = smx.tile([128, 2, 512], BF16, tag="sfb", name=f"mx{h}_{c}")
                    nc.vector.tensor_max(mx[:], s_h[0][:, dsl, :],
                                         s_h[1][:, dsl, :])
                    nc.gpsimd.dma_start(mx_in[h].ap()[:, dsl, :], mx[:])

            def exp_and_sum(h, s_h, p_h):
                # after AR-max(h): subtract gmax, exp -> p bf16, local sum
                for c in range(ARC):
                    dsl = slice(2 * c, 2 * c + 2)
                    gmxb = smx.tile([128, 2, 512], BF16, tag="sfb",
                                    name=f"gmxb{h}_{c}")
                    nc.gpsimd.dma_start(gmxb[:], mx_out[h].ap()[:, dsl, :])
                    gmx = smx.tile([128, 2, 512], F32, tag="sff",
                                   name=f"gmx{h}_{c}")
                    nc.vector.tensor_copy(gmx[:], gmxb[:])
                    for b in range(BL):
                        nc.vector.tensor_sub(s_h[b][:, dsl, :],
                                             s_h[b][:, dsl, :], gmx[:])
                        nc.scalar.activation(p_h[b][:, dsl, :],
                                             s_h[b][:, dsl, :], AF.Exp)
                    sm = smx.tile([128, 2, 512], BF16, tag="sfb", name=f"sm{h}_{c}")
                    nc.vector.tensor_add(sm[:], p_h[0][:, dsl, :],
                                         p_h[1][:, dsl, :])
                    nc.gpsimd.dma_start(sm_in[h].ap()[:, dsl, :], sm[:])

            def attn_mul(h, p_h):
                # after AR-sum(h): 1/Z = exp(-ln(Z)); attn = p * rec in place
                for c in range(ARC):
                    dsl = slice(2 * c, 2 * c + 2)
                    zz = smx.tile([128, 2, 512], BF16, tag="sfb", name=f"zz{h}_{c}")
                    nc.gpsimd.dma_start(zz[:], sm_out[h].ap()[:, dsl, :])
                    rec = smx.tile([128, 2, 512], F32, tag="sff",
                                   name=f"rec{h}_{c}")
                    nc.scalar.activation(rec[:], zz[:], AF.Ln)
                    recb = smx.tile([128, 2, 512], BF16, tag="sfb",
                                    name=f"recb{h}_{c}")
                    nc.scalar.activation(recb[:], rec[:], AF.Exp, scale=-1.0)
                    for b in range(BL):
                        nc.vector.tensor_mul(p_h[b][:, dsl, :],
                                             p_h[b][:, dsl, :], recb[:])

            def stage_c_round(h, attn_h, b, mg):
                # kc-outer / m-inner over 8 PSUM banks; v^T via xbar JIT
                vts = []
                for j in range(8):
                    m = mg + j
                    vt = ktp.tile([128, KC, 128], BF16, tag="kt",
                                  name=f"vt{h}_{b}_{m}")
                    nc.sync.dma_start(
                        vt[:], v_bf.ap()[b, m * 128:(m + 1) * 128, :],
                        transpose=True)
                    vts.append(vt)
                pss = [ps8.tile([128, 512], F32, tag=f"ps{j}",
                                name=f"cps{h}_{b}_{mg}_{j}") for j in range(8)]
                for kc in range(KC):
                    for j in range(8):
                        nc.tensor.matmul(
                            pss[j][:], vts[j][:, kc, :], attn_h[b][:, kc, :],
                            start=(kc == 0), stop=(kc == KC - 1))
                for j in range(8):
                    m = mg + j
                    ost = bp.tile([128, 512], F32, tag="ost",
                                  name=f"ost{h}_{b}_{m}")
                    nc.vector.tensor_copy(ost[:], pss[j][:])
                    nc.scalar.dma_start(
                        out2.ap()[b, m * 128:(m + 1) * 128,
                                  h * 512:(h + 1) * 512], ost[:])

            # ---- pipelined execution ----
            p_t = {}
            for h in range(EH):
                p_t[h] = [sfp.tile([128, KC, 512], BF16, tag=f"p{b}",
                                   name=f"p{h}_{b}") for b in range(BL)]

            t0 = stage_a(0)
            s0 = stage_b(0, t0)
            local_max(0, s0)
            ar_mx0 = nc.gpsimd.collective_compute(
                "AllReduce", mybir.AluOpType.max, replica_groups=RG,
                ins=[mx_in[0].ap().opt()], outs=[mx_out[0].ap().opt()])
            tile.add_dep_helper(ar_mx0.ins, ar_w2.ins, sync=False,
                                reason="serialize collectives")

            t1 = stage_a(1)            # PE busy while AR-max(h0) flies
            exp_and_sum(0, s0, p_t[0])
            ar_sm0 = nc.gpsimd.collective_compute(
                "AllReduce", mybir.AluOpType.add, replica_groups=RG,
                ins=[sm_in[0].ap().opt()], outs=[sm_out[0].ap().opt()])
            tile.add_dep_helper(ar_sm0.ins, ar_mx0.ins, sync=False,
                                reason="serialize collectives")

            s1 = stage_b(1, t1)        # PE busy while AR-sum(h0) flies
            attn_mul(0, p_t[0])
            local_max(1, s1)
            ar_mx1 = nc.gpsimd.collective_compute(
                "AllReduce", mybir.AluOpType.max, replica_groups=RG,
                ins=[mx_in[1].ap().opt()], outs=[mx_out[1].ap().opt()])
            tile.add_dep_helper(ar_mx1.ins, ar_sm0.ins, sync=False,
                                reason="serialize collectives")

            stage_c_round(0, p_t[0], 0, 0)   # PE busy while AR-max(h1) flies
            stage_c_round(0, p_t[0], 0, 8)
            exp_and_sum(1, s1, p_t[1])
            ar_sm1 = nc.gpsimd.collective_compute(
                "AllReduce", mybir.AluOpType.add, replica_groups=RG,
                ins=[sm_in[1].ap().opt()], outs=[sm_out[1].ap().opt()])
            tile.add_dep_helper(ar_sm1.ins, ar_mx1.ins, sync=False,
                                reason="serialize collectives")
            stage_c_round(0, p_t[0], 1, 0)
            stage_c_round(0, p_t[0], 1, 8)
            attn_mul(1, p_t[1])

            # bridge the AR-sum(h1) wait; keeps the PE HAM clock warm
            dps = ps8.tile([128, 512], F32, tag="ps0", name="dummy_ps")
            for i in range(N_DUMMY):
                nc.tensor.matmul(
                    dps[:], t1[0][:, 0, 0:128], t1[1][:, i % M_T, :],
                    start=True, stop=True)

            for b in range(BL):
                for mg in (0, 8):
                    stage_c_round(1, p_t[1], b, mg)

            bp_cm.__exit__(None, None, None)
            sfp_cm.__exit__(None, None, None)
            sm_cm.__exit__(None, None, None)
            ktp_cm.__exit__(None, None, None)
            ps8_cm.__exit__(None, None, None)
            tp_cm.__exit__(None, None, None)

    nc.compile()
    return nc


_NC = None


def _get_nc():
    global _NC
    if _NC is None:
        _NC = build()
    return _NC


def kernel(q, k, v, W, U):
    q = np.ascontiguousarray(np.asarray(q, dtype=np.float32))
    k = np.ascontiguousarray(np.asarray(k, dtype=np.float32))
    v = np.ascontiguousarray(np.asarray(v, dtype=np.float32))
    W = np.ascontiguousarray(np.asarray(W, dtype=np.float32))
    U = np.ascontiguousarray(np.asarray(U, dtype=np.float32))

    nc = _get_nc()
    in_maps = [
        {
            "q2": q[c * BL:(c + 1) * BL],
            "k2": k[c * BL:(c + 1) * BL],
            "v2": v[c * BL:(c + 1) * BL],
            "W": W,
            "U": U,
        }
        for c in range(N_CORES)
    ]
    res = run_bass_kernel_spmd(nc, in_maps, core_ids=list(range(N_CORES)))
    out = np.concatenate([res.results[c]["out"] for c in range(N_CORES)], axis=0)
    return out.astype(np.float32)


if __name__ == "__main__":
    rng = np.random.default_rng(0)
    q = rng.standard_normal((B, D), dtype=np.float32)
    k = rng.standard_normal((B, S, D), dtype=np.float32)
    v = rng.standard_normal((B, S, D), dtype=np.float32)
    W = (rng.standard_normal((D, D), dtype=np.float32) / np.sqrt(D)).astype(np.float32)
    U = (rng.standard_normal((D, D), dtype=np.float32) / np.sqrt(D)).astype(np.float32)
    out = kernel(q=q, k=k, v=v, W=W, U=U)
    print("out", out.shape, out.dtype, float(np.abs(out).mean()))
